# revision 43
# baseline (speedup 1.0000x reference)
"""Trainium2 Bass kernel for nn_AgentGnn (2-layer CGConv GNN, 128 scenes x 64 agents).

Structure exploited:
- Edges are fully-connected per 64-agent scene (no self loops), so gather/scatter
  becomes dense 64x64 blocks: agg[i] = sum_j sigmoid(F_ij) * softplus(S_ij).
- Per-edge linear terms factor into per-node terms:
    F_ij = af[i] + bf[j],  af = x_i @ Wf[:D] + c_i @ Wf[2D:],
                           bf = x_j @ Wf[D:2D] - c_j @ Wf[2D:]  (+bias)
- sigmoid(F) on ACT from the PE pairwise sums (indicator matmul); scenes are
  processed in table-phase groups with the ACT program order pinned so the
  two activation-table sets (sigmoid / ln+exp) batch their loads.
- softplus(S) = ln(1+e^S) with e^S = e^{as_i} * e^{bs_j}: per-NODE exps
  (tiny ACT passes on weight-stationary matmul outputs in [feature, node]
  layout) combined by one double-broadcast pairwise product per scene -
  mostly on gpsimd (0.42 eff), 1/4 of scenes on DVE (1x broadcast) to
  balance the engines. exp(S) <= ~700 here, far below f16 max. Only ONE
  pairwise ACT pass (Ln) remains on the S side; m = sg * pch on DVE (2x).
- ACT is the bottleneck engine (~81% busy): 2 table passes per pairwise
  element is the floor for this math (no table set holds both a sigmoid-
  family fn and ln; softplus/mish tables don't exist in this overlay).
- j-reduction: all-TT f16 2x halving fold chain (no 1x tensor_reduce).
- The diagonal (self-edge) is removed analytically: for i=j the center
  terms cancel, so F_ii = x@(Wfd+Wfs)+bf (one tiny per-batch matmul) and
  e^{S_ii} = esa*esb; m_ii is subtracted after the j-reduction. No strided
  element accesses anywhere (single-element-stride ops cost 30-150us/op on
  real HW).
- BatchNorm uses exact full stats (per-core/subset stats measure 4-8e-2
  rel err - over the 2e-2 gate); the [128,2] AllReduce per layer is on the
  critical path, so the LAYER TAIL is squeezed: a 3-scene final group,
  halved Ln + halved fold chains there, per-group stats partials with the
  last scene's stats alone, and a fused BN-consts chain.
- Startup: packed weight DMAs (3/layer instead of 12; serial dma_start
  issue on SP costs ~565ns each), xt in halves, batch-1 node terms
  deferred past group 0, and a dummy Exp to hoist the first table load.
- Sharding: 16 scenes (1024 nodes) per core, pure data parallel otherwise.

TimelineSim (single core, fake collective): 327.7us vs 349.6us for the
previous version. Measured on HW (M17 burst marginal): the per-exec fixed
overhead of this runtime path is ~350us on top of the kernel itself
(unroll=2 experiment: marginal 1.09-1.14ms vs 0.74ms at unroll=1).
"""

import numpy as np

N_SAMPLES = 128
AGENTS = 64
D = 128
EDIM = 2
N = N_SAMPLES * AGENTS
EPS = 1e-5

N_CORES = 8
SCENES_PC = N_SAMPLES // N_CORES      # 16 scenes per core
NODES_PC = SCENES_PC * AGENTS         # 1024 nodes per core
PAIR = AGENTS * AGENTS                # 4096 pairwise cols per scene
CHUNK = 2048                          # pairwise chunk (32 i x 64 j)
N_CHUNKS = PAIR // CHUNK              # 2
SUB = 8                               # scenes per node-matmul batch
GRP = 4                               # scenes per ACT-table phase group
DVE_BT_MOD = 4                        # s % DVE_BT_MOD == 0 -> product on DVE
                                      # (gpsimd mult runs at ~0.42 eff; split
                                      # the pairwise e^S products to balance)

_CACHE: dict = {}


def _build_indicator() -> np.ndarray:
    ind = np.zeros((128, PAIR), np.float32)
    for i in range(AGENTS):
        ind[i, i * AGENTS:(i + 1) * AGENTS] = 1.0
    for j in range(AGENTS):
        ind[64 + j, j::AGENTS] = 1.0
    return ind


def _expected_edges():
    a = np.arange(AGENTS)
    rows = np.repeat(a, AGENTS)
    cols = np.tile(a, AGENTS)
    mask = rows != cols
    rows, cols = rows[mask], cols[mask]
    offsets = (np.arange(N_SAMPLES) * AGENTS)[:, None]
    src = (rows[None, :] + offsets).ravel().astype(np.int32)
    dst = (cols[None, :] + offsets).ravel().astype(np.int32)
    return src, dst


def _numpy_fallback(gnn_in, centers, edge_src, edge_dst, ws):
    # generic (slow) reference path, used only if edges don't match the
    # expected block-diagonal fully-connected pattern
    def sigmoid(x):
        return 1.0 / (1.0 + np.exp(-x))

    def softplus(x):
        return np.logaddexp(0.0, x)

    x = gnn_in.astype(np.float64)
    e = (centers[edge_dst] - centers[edge_src]).astype(np.float64)
    for li in (1, 2):
        Wf, bf, Ws, bs, gamma, beta = (ws[f"Wf{li}"], ws[f"bf{li}"], ws[f"Ws{li}"],
                                       ws[f"bs{li}"], ws[f"gamma{li}"], ws[f"beta{li}"])
        z = np.concatenate([x[edge_dst], x[edge_src], e], axis=-1)
        m = sigmoid(z @ Wf + bf) * softplus(z @ Ws + bs)
        agg = np.zeros((N, D))
        np.add.at(agg, edge_dst, m)
        mu = agg.mean(axis=0)
        var = agg.var(axis=0)
        agg = (agg - mu) / np.sqrt(var + EPS) * gamma + beta
        x = np.maximum(agg + x, 0.0)
    return x.astype(np.float32)


def _build_nc(use_collectives=True, unroll=1, stats_subset=False,
              stat_divisor=None, use_gpsimd=True, debug_plain=False,
              palin=False, bt_mod=DVE_BT_MOD, bufs=None, g0sz=4):
    import concourse.bacc as bacc
    import concourse.mybir as mybir
    import concourse.tile as tile

    f32 = mybir.dt.float32
    f16 = mybir.dt.float16
    AF = mybir.ActivationFunctionType
    OP = mybir.AluOpType

    nc = bacc.Bacc("TRN2", target_bir_lowering=False, debug=False,
                   num_devices=N_CORES if use_collectives else 1)

    # ---- I/O ----
    xt_in = nc.dram_tensor("xt", [D, NODES_PC], f32, kind="ExternalInput")
    ct_in = nc.dram_tensor("ct", [EDIM, NODES_PC], f16, kind="ExternalInput")
    ind_in = nc.dram_tensor("ind", [128, PAIR], f16, kind="ExternalInput")
    win = {}
    for li in (1, 2):
        # packed weights: one DMA per pack instead of 12 (startup is gated
        # by serial dma_start issue on the SP sequencer, ~565ns each)
        win[f"wp{li}"] = nc.dram_tensor(f"wp{li}", [D, 4 * D], f16, kind="ExternalInput")
        win[f"we{li}"] = nc.dram_tensor(f"we{li}", [EDIM, 4 * D], f16, kind="ExternalInput")
        win[f"bp{li}"] = nc.dram_tensor(f"bp{li}", [D, 4], f32, kind="ExternalInput")
    out_t = nc.dram_tensor("out_t", [D, NODES_PC], f32, kind="ExternalOutput")

    # nodes the BN statistics are taken over (global, across all cores)
    n_stat = stat_divisor or ((N // 2) if stats_subset else N)

    acts = []  # ACT instructions whose engine order we pin (table batching)

    def act(*args, **kwargs):
        inst = nc.scalar.activation(*args, **kwargs)
        acts.append(inst)
        return inst

    nb = dict(chk=2, pch=5, bt=5, sg=6, ps=2)
    if bufs:
        nb.update(bufs)
    with tile.TileContext(nc) as tc:
        with (
            tc.tile_pool(name="cst", bufs=1) as cst,
            tc.tile_pool(name="wrk", bufs=1) as wrk,
            tc.tile_pool(name="chk", bufs=nb["chk"]) as chk,
            tc.tile_pool(name="pchp", bufs=nb["pch"]) as pchp,
            tc.tile_pool(name="btp", bufs=nb["bt"]) as btp,
            tc.tile_pool(name="sgp", bufs=nb["sg"]) as sgp,
            tc.tile_pool(name="ps", bufs=nb["ps"], space="PSUM") as ps,
            tc.tile_pool(name="dram", bufs=1, space="DRAM") as dram,
        ):
            # ---- load constants ----
            xt = cst.tile([D, NODES_PC], f32)
            ct = cst.tile([EDIM, NODES_PC], f16)
            ind = cst.tile([128, PAIR], f16)
            nc.sync.dma_start(xt[:, 0:NODES_PC // 2], xt_in.ap()[:, 0:NODES_PC // 2])
            nc.sync.dma_start(xt[:, NODES_PC // 2:], xt_in.ap()[:, NODES_PC // 2:])
            wt = {}
            # issue order: layer-1 weights first (startup critical path),
            # then ct/ind (needed at first pairwise matmuls), then layer 2
            for k in ("wp1", "we1", "bp1"):
                h = win[k]
                t = cst.tile(list(h.shape), h.dtype, name=f"t_{k}", tag=f"t_{k}")
                nc.sync.dma_start(t[:], h.ap())
                wt[k] = t
            nc.sync.dma_start(ct[:], ct_in.ap())
            nc.sync.dma_start(ind[:, 0:512], ind_in.ap()[:, 0:512])
            nc.sync.dma_start(ind[:, 512:], ind_in.ap()[:, 512:])
            for k in ("wp2", "we2", "bp2"):
                h = win[k]
                t = cst.tile(list(h.shape), h.dtype, name=f"t_{k}", tag=f"t_{k}")
                nc.sync.dma_start(t[:], h.ap())
                wt[k] = t

            # dummy Exp depending only on the tiny bp1 DMA: hoists the first
            # ln/exp ACT table load to ~t=1us instead of gluing it to the
            # first real exp (which waits on node matmuls)
            warm = wrk.tile([D, 1], f32, name="warm", tag="warm")
            act(warm[:], wt["bp1"][:, 0:1], AF.Exp, bias=0.0, scale=1.0)

            HALF = NODES_PC // 2
            x_carry = xt
            for rep in range(unroll):
              # chain reps through x_carry so unrolled timing builds aren't
              # dead-code eliminated (rep 0 reads the real input)
              x_cur = x_carry
              # fp16 copy of x for the node matmuls (residual stays fp32);
              # done in halves so compute can start before the full DMA
              x16 = wrk.tile([D, NODES_PC], f16, name=f"x16_0r{rep}", tag="x16_0")
              nc.vector.tensor_copy(x16[:, 0:HALF], x_cur[:, 0:HALF])
              nc.vector.tensor_copy(x16[:, HALF:], x_cur[:, HALF:])
              for li0 in (1, 2):
                li = f"{li0}" if rep == 0 else f"{li0}r{rep}"
                par = li0 % 2
                wli = li0
                wp, we, bp = wt[f"wp{wli}"], wt[f"we{wli}"], wt[f"bp{wli}"]
                wfd, wfs = wp[:, 0:D], wp[:, D:2 * D]
                wsd, wss = wp[:, 2 * D:3 * D], wp[:, 3 * D:4 * D]
                wfe, wfen = we[:, 0:D], we[:, D:2 * D]
                wse, wsen = we[:, 2 * D:3 * D], we[:, 3 * D:4 * D]
                bf, bs = bp[:, 0:1], bp[:, 1:2]
                ga, be = bp[:, 2:3], bp[:, 3:4]

                wfds = wrk.tile([D, D], f16, name=f"wfds{li}", tag=f"wfds{par}")
                nc.vector.tensor_tensor(wfds[:], wfd, wfs, OP.add)
                agg = wrk.tile([D, NODES_PC], f32, name=f"agg{li}", tag=f"agg{par}")
                stats = wrk.tile([D, 2], f32, name=f"stats{li}", tag=f"stats{par}")
                # group boundaries: a 3-scene tail group shortens the DVE
                # reduction backlog ahead of the BN stats collective
                GROUPS = [(0, g0sz), (g0sz, 8), (8, 13), (13, 16)]
                NGRP = len(GROUPS)
                GMAX = max(g1 - g0 for g0, g1 in GROUPS)
                sscr = wrk.tile([D, GMAX * AGENTS], f32, name=f"sscr{li}",
                                tag=f"sscr{par}")
                # per-group partials, no serial accumulation chain; one extra
                # slot pair for the split last scene
                sall = wrk.tile([D, 2 * (NGRP + 1)], f32, name=f"sall{li}",
                                tag=f"sall{par}")

                abf = {}
                esa_t, esb_t = {}, {}

                def node_f(batch, bi):
                    # per-scene F-path node terms in [node, feature] layout
                    # (lhsT operand of the pairwise indicator matmul); scenes
                    # pairwise-share a PSUM tile so the first scene's copy
                    # lands after 8 (not 32) matmuls - the layer-boundary
                    # critical path runs through scene 0's abf
                    for k0 in range(0, len(batch), 2):
                        pair = batch[k0:k0 + 2]
                        pab = ps.tile([128, len(pair) * D], f32,
                                      name=f"pab{bi}_{k0}_{li}", tag="pp")
                        for k, s in enumerate(pair):
                            xs = x16[:, s * AGENTS:(s + 1) * AGENTS]
                            cs = ct[:, s * AGENTS:(s + 1) * AGENTS]
                            o = k * D
                            nc.tensor.matmul(pab[0:64, o:o + D], lhsT=cs, rhs=wfe, start=True, stop=False)
                            nc.tensor.matmul(pab[0:64, o:o + D], lhsT=xs, rhs=wfd, start=False, stop=True)
                            nc.tensor.matmul(pab[64:128, o:o + D], lhsT=cs, rhs=wfen, start=True, stop=False)
                            nc.tensor.matmul(pab[64:128, o:o + D], lhsT=xs, rhs=wfs, start=False, stop=True)
                        abt = wrk.tile([128, len(pair) * D], f16,
                                       name=f"ab{bi}_{k0}_{li}", tag=f"ab{bi}_{k0}")
                        nc.vector.tensor_copy(abt[:], pab[:])
                        for k, s in enumerate(pair):
                            abf[s] = abt[:, k * D:(k + 1) * D]

                def node_s(batch, bi):
                    # batched S-path node terms in [feature, node] layout
                    # (weight-stationary), then per-node exp on ACT
                    n0 = batch[0] * AGENTS
                    n = len(batch) * AGENTS
                    pn = ps.tile([128, 2 * n], f32, name=f"pn{bi}_{li}", tag="pp")
                    nc.tensor.matmul(pn[:, 0:n], lhsT=wsd, rhs=x16[:, n0:n0 + n],
                                     start=True, stop=False)
                    nc.tensor.matmul(pn[:, 0:n], lhsT=wse, rhs=ct[:, n0:n0 + n],
                                     start=False, stop=True)
                    nc.tensor.matmul(pn[:, n:2 * n], lhsT=wss, rhs=x16[:, n0:n0 + n],
                                     start=True, stop=False)
                    nc.tensor.matmul(pn[:, n:2 * n], lhsT=wsen, rhs=ct[:, n0:n0 + n],
                                     start=False, stop=True)
                    ea = wrk.tile([D, n], f16, name=f"esa{bi}_{li}", tag=f"esa{bi % 2}")
                    eb = wrk.tile([D, n], f16, name=f"esb{bi}_{li}", tag=f"esb{bi % 2}")
                    act(ea[:], pn[:, 0:n], AF.Exp, bias=bs, scale=1.0)
                    act(eb[:], pn[:, n:2 * n], AF.Exp, bias=0.0, scale=1.0)
                    esa_t[bi] = ea
                    esb_t[bi] = eb
                    # self-edge terms: F_ii = x@(Wfd+Wfs)+bf (centers cancel),
                    # e^{S_ii} = esa*esb. sigmoid/ln are scheduled into the
                    # matching table phases; diag is subtracted after reduce.
                    pnf = ps.tile([128, n], f32, name=f"pnf{bi}_{li}", tag="pp")
                    nc.tensor.matmul(pnf[:, 0:n], lhsT=wfds[:], rhs=x16[:, n0:n0 + n],
                                     start=True, stop=True)
                    # free the PSUM slot right away (sigmoid happens much later);
                    # on DVE to keep the (bottleneck) ACT engine clear
                    fsum = wrk.tile([D, n], f16, name=f"fsum{bi}_{li}", tag=f"fsum{bi % 2}")
                    nc.vector.tensor_copy(fsum[:], pnf[:, 0:n])
                    eii = wrk.tile([D, n], f16, name=f"eii{bi}_{li}", tag=f"eii{bi % 2}")
                    nc.vector.tensor_tensor(eii[:], ea[:], eb[:], OP.mult)
                    dparts[bi] = (fsum, eii)

                sg_map, pch_map = {}, {}
                dparts, dmt = {}, {}

                def diag_sig(bi):
                    # sigmoid(F_ii) - rides in the sigmoid table phase
                    fsum, eii = dparts[bi]
                    n = fsum.shape[1]
                    sii = wrk.tile([D, n], f16, name=f"sii{bi}_{li}", tag=f"sii{bi % 2}")
                    act(sii[:], fsum[:], AF.Sigmoid, bias=bf, scale=1.0)
                    dparts[bi] = (sii, eii)

                def diag_ln(bi):
                    # ln(1+e^{S_ii}) - rides in the ln/exp table phase
                    sii, eii = dparts[bi]
                    n = eii.shape[1]
                    spd = wrk.tile([D, n], f16, name=f"spd{bi}_{li}", tag=f"spd{bi % 2}")
                    act(spd[:], eii[:], AF.Ln, bias=1.0, scale=1.0)
                    dparts[bi] = (sii, spd)

                def diag_dm(bi):
                    # m_ii = sigmoid(F_ii) * ln(1+e^{S_ii}) on DVE
                    sii, spd = dparts[bi]
                    n = spd.shape[1]
                    dm = wrk.tile([D, n], f16, name=f"dm{bi}_{li}", tag=f"dm{bi % 2}")
                    nc.vector.tensor_tensor(dm[:], sii[:], spd[:], OP.mult)
                    dmt[bi] = dm

                def scene_sig(s):
                    # F-path: pairwise sums on PE, sigmoid on ACT
                    sg = sgp.tile([D, PAIR], f16, name=f"sg{s}", tag="sg")
                    for c in range(N_CHUNKS):
                        pf = ps.tile([D, CHUNK], f32, name=f"pf{s}_{c}", tag="pp")
                        for k in range(CHUNK // 512):
                            col = c * CHUNK + k * 512
                            nc.tensor.matmul(pf[:, k * 512:(k + 1) * 512],
                                             lhsT=abf[s][:],
                                             rhs=ind[:, col:col + 512],
                                             start=True, stop=True)
                        act(sg[:, c * CHUNK:(c + 1) * CHUNK], pf[:],
                            AF.Sigmoid, bias=bf, scale=1.0)
                    sg_map[s] = sg

                def scene_bt(s, bi):
                    ea, eb = esa_t[bi], esb_t[bi]
                    o = (s % SUB) * AGENTS
                    ea_b = ea[:, o:o + AGENTS].unsqueeze(2).broadcast_to(
                        (D, AGENTS, AGENTS))
                    eb_b = eb[:, o:o + AGENTS].unsqueeze(1).broadcast_to(
                        (D, AGENTS, AGENTS))
                    # pairwise e^S = e^{as_i} * e^{bs_j} (diag handled
                    # analytically afterwards); split across DVE (1x, broadcast
                    # APs) and gpsimd (0.42 eff) to balance the two engines
                    bt = btp.tile([D, PAIR], f16, name="bt", tag="bt")
                    bt3 = bt.rearrange("p (i j) -> p i j", j=AGENTS)
                    if use_gpsimd and s % bt_mod != 2 % bt_mod:
                        nc.gpsimd.tensor_tensor(bt3, ea_b, eb_b, OP.mult)
                    else:
                        nc.vector.tensor_tensor(bt3, ea_b, eb_b, OP.mult)
                    return bt

                LAST = SCENES_PC - 1

                def scene_ln(s, bt):
                    # softplus = ln(1 + e^S) - the only pairwise S-path ACT op.
                    # The LAST scene of the layer is split into i-halves so the
                    # DVE reduction (critical path into the BN collective)
                    # starts ~2us earlier.
                    pch = pchp.tile([D, PAIR], f16, name="pch", tag="pch")
                    if s >= GROUPS[-1][0]:
                        act(pch[:, 0:PAIR // 2], bt[:, 0:PAIR // 2],
                            AF.Ln, bias=1.0, scale=1.0)
                        act(pch[:, PAIR // 2:], bt[:, PAIR // 2:],
                            AF.Ln, bias=1.0, scale=1.0)
                    else:
                        act(pch[:], bt[:], AF.Ln, bias=1.0, scale=1.0)
                    pch_map[s] = pch

                def red_part(s, bi, pch, sg, h, nh):
                    # m = sigmoid(F) * softplus(S) on DVE (f16 2x), then
                    # j-reduction: all-TT halving fold chain (f16 2x; avoids
                    # the 1x-rate tensor_reduce). h/nh select an i-chunk.
                    ni = AGENTS // nh
                    sl = slice(h * ni * AGENTS, (h + 1) * ni * AGENTS)
                    nc.vector.tensor_tensor(pch[:, sl], sg[:, sl], pch[:, sl],
                                            OP.mult)
                    cur = pch[:, sl].rearrange("p (i j) -> p i j", j=AGENTS)
                    w = AGENTS
                    fi = 0
                    while w > 2:
                        w //= 2
                        fi += 1
                        fd = chk.tile([D, ni * w], f16, name=f"fd{fi}",
                                      tag=f"fd{fi}")
                        f3 = fd.rearrange("p (i j) -> p i j", j=w)
                        nc.vector.tensor_tensor(f3, cur[:, :, 0:w], cur[:, :, w:2 * w],
                                                OP.add)
                        cur = f3
                    asl = agg[:, s * AGENTS + h * ni:s * AGENTS + (h + 1) * ni]
                    nc.vector.tensor_tensor(
                        asl.rearrange("p (i j) -> p i j", j=1),
                        cur[:, :, 0:1], cur[:, :, 1:2], OP.add)
                    o = (s % SUB) * AGENTS + h * ni
                    nc.vector.tensor_tensor(
                        asl, asl, dmt[bi][:, o:o + ni], OP.subtract)

                def stats_part(slot, n0, n1):
                    # BN partial stats (sum + sumsq) over node cols [n0, n1)
                    gsl = agg[:, n0:n1]
                    nc.vector.tensor_tensor(sscr[:, 0:n1 - n0], gsl, gsl, OP.mult)
                    nc.vector.tensor_reduce(sall[:, 2 * slot:2 * slot + 1], gsl,
                                            axis=mybir.AxisListType.X, op=OP.add)
                    nc.vector.tensor_reduce(sall[:, 2 * slot + 1:2 * slot + 2],
                                            sscr[:, 0:n1 - n0],
                                            axis=mybir.AxisListType.X, op=OP.add)

                def scene_red(s, bi, gi, g0, g1):
                    pch = pch_map.pop(s)
                    sg = sg_map.pop(s)
                    if gi == NGRP - 1:
                        # tail group: halved reductions overlap the (also
                        # halved) Ln ops - shortest chain into the collective
                        red_part(s, bi, pch, sg, 0, 2)
                        red_part(s, bi, pch, sg, 1, 2)
                    else:
                        red_part(s, bi, pch, sg, 0, 1)
                    if s == LAST:
                        # last scene's stats alone (the rest of its group was
                        # already folded in at s-1) - shortest possible tail
                        stats_part(NGRP, s * AGENTS, (s + 1) * AGENTS)
                    elif s == g1 - 1 and g1 != SCENES_PC:
                        stats_part(gi, g0 * AGENTS, g1 * AGENTS)
                    elif s == LAST - 1:
                        # tail group's scenes ahead of the last scene
                        stats_part(gi, g0 * AGENTS, (s + 1) * AGENTS)

                def bn_consts():
                    # combine per-group partials: view [d, c(2), g] with
                    # c outer (stride 1) and g inner (stride 2), reduce g
                    sview = sall[:, 0:2 * (NGRP + 1)].rearrange(
                        "p (s c) -> p c s", c=2)
                    nc.vector.tensor_reduce(
                        stats.rearrange("p (c o) -> p c o", o=1), sview,
                        axis=mybir.AxisListType.X, op=OP.add)
                    # AllReduce the [sum, sumsq] stats, then fold into A, B
                    cc_in = dram.tile([D, 2], f32, name=f"ccin{li}", tag=f"ccin{li}")
                    cc_out = dram.tile([D, 2], f32, name=f"ccout{li}", tag=f"ccout{li}",
                                       addr_space="Shared")
                    nc.sync.dma_start(cc_in[:], stats[:])
                    if use_collectives:
                        nc.gpsimd.collective_compute(
                            "AllReduce", OP.add,
                            replica_groups=[list(range(N_CORES))],
                            ins=[cc_in.opt()], outs=[cc_out.opt()])
                    else:
                        nc.sync.dma_start(cc_out[:], cc_in[:])
                    stot = wrk.tile([D, 2], f32, name=f"stot{li}", tag="stot")
                    nc.sync.dma_start(stot[:], cc_out[:])
                    # me = [mu, ex2]; rstd = exp(-0.5*ln(var+eps));
                    # A = gamma*rstd; B = beta - mu*A (shortest serial chain)
                    me = wrk.tile([D, 2], f32, name="me", tag="me")
                    nc.vector.tensor_scalar_mul(me[:], stot[:], 1.0 / n_stat)
                    var = wrk.tile([D, 1], f32, name="var", tag="var")
                    nc.vector.tensor_tensor(var[:], me[:, 0:1], me[:, 0:1], OP.mult)
                    nc.vector.scalar_tensor_tensor(var[:], me[:, 1:2], EPS,
                                                   var[:], OP.add, OP.subtract)
                    rstd = wrk.tile([D, 1], f32, name="rstd", tag="rstd")
                    nc.scalar.activation(rstd[:], var[:], AF.Ln, bias=0.0, scale=1.0)
                    nc.scalar.activation(rstd[:], rstd[:], AF.Exp, bias=0.0, scale=-0.5)
                    A = wrk.tile([D, 1], f32, name="A", tag="A")
                    Bt = wrk.tile([D, 1], f32, name="Bt", tag="Bt")
                    nc.vector.tensor_tensor(A[:], ga, rstd[:], OP.mult)
                    nc.vector.tensor_tensor(Bt[:], me[:, 0:1], A[:], OP.mult)
                    nc.vector.tensor_tensor(Bt[:], be, Bt[:], OP.subtract)
                    return A, Bt

                batches = [list(range(b0, min(b0 + SUB, SCENES_PC)))
                           for b0 in range(0, SCENES_PC, SUB)]
                node_s(batches[0], 0)
                node_f(batches[0], 0)
                ar_done = [False]

                def run_group(gi, g0, g1):
                    # palindromic ACT phase order: even groups run
                    # [sigmoid-table phase, ln-table phase], odd groups
                    # [ln, sigmoid] - adjacent same-set phases merge, so
                    # table loads drop from 2/group to ~1/group. The ln
                    # phase never depends on the sigmoid phase (the m
                    # product on DVE joins them afterwards).
                    bts = {s: scene_bt(s, s // SUB) for s in range(g0, g1)}
                    dbis = [bi for bi in (0, 1) if g0 <= bi * SUB < g1]

                    def sig_phase():
                        for bi in dbis:
                            diag_sig(bi)
                        for s in range(g0, g1):
                            scene_sig(s)

                    def ln_phase():
                        for bi in dbis:
                            diag_ln(bi)
                        for s in range(g0, g1):
                            scene_ln(s, bts[s])

                    if (gi % 2 == 0) or not palin:
                        sig_phase()
                        ln_phase()
                    else:
                        ln_phase()
                        sig_phase()
                    for bi in dbis:
                        diag_dm(bi)
                    for s in range(g0, g1):
                        scene_red(s, s // SUB, gi, g0, g1)

                for gi, (g0, g1) in enumerate(GROUPS):
                    if g0 == SUB:
                        # batch-1 node terms deferred past the early groups:
                        # keeps the two PSUM slots free during startup
                        # (batch-0 exps release them before group 0's
                        # pairwise matmuls), and the Exp ops ride the
                        # adjacent ln-set phase
                        node_s(batches[1], 1)
                        node_f(batches[1], 1)
                    run_group(gi, g0, g1)

                # x_next = relu(agg*A + B + x_cur) applied per half so the
                # next layer's node matmuls (which only need one half) can
                # start while this layer's second half is still draining
                xn = wrk.tile([D, NODES_PC], f32, name=f"x{li}", tag=f"xn{par}")
                x16n = None
                if li0 == 1 or rep + 1 < unroll:
                    x16n = wrk.tile([D, NODES_PC], f16, name=f"x16_{li}",
                                    tag=f"x16_{li0 % 2}")

                def apply_half(h):
                    sl = slice(h * HALF, (h + 1) * HALF)
                    nc.vector.scalar_tensor_tensor(xn[:, sl], agg[:, sl],
                                                   A[:, 0:1], x_cur[:, sl],
                                                   OP.mult, OP.add)
                    nc.vector.tensor_scalar(xn[:, sl], xn[:, sl], Bt[:, 0:1],
                                            0.0, OP.add, OP.max)
                    if x16n is not None:
                        nc.vector.tensor_copy(x16n[:, sl], xn[:, sl])

                A, Bt = bn_consts()
                apply_half(0)
                apply_half(1)
                x_cur = xn
                x16 = x16n
              x_carry = x_cur

            nc.sync.dma_start(out_t.ap()[:, 0:HALF], x_cur[:, 0:HALF])
            nc.sync.dma_start(out_t.ap()[:, HALF:], x_cur[:, HALF:])

        from concourse.tile_rust import add_dep_helper
        for a, b in zip(acts, acts[1:]):
            add_dep_helper(b.ins, a.ins, sync=False,
                           reason="ACT table-set batching order")

    # Restrict the act-table chooser so Exp and Ln resolve to the shared
    # natural_log_exp set (all ACT ops here live in that one set; the
    # default chooser could otherwise alternate sets and thrash ~2.7us
    # table loads).
    keep = {"sigmoid_and_others", "natural_log_exp_and_others"}
    orig_tables = bacc.get_activation_tables

    def patched_tables(arch):
        return {k: (v if k in keep else set())
                for k, v in orig_tables(arch).items()}

    bacc.get_activation_tables = patched_tables
    try:
        nc.compile()
    finally:
        bacc.get_activation_tables = orig_tables
    return nc


def _get_nc():
    if "nc" not in _CACHE:
        _CACHE["nc"] = _build_nc()
    return _CACHE["nc"]


def kernel(**inputs) -> np.ndarray:
    gnn_in = np.ascontiguousarray(np.asarray(inputs["gnn_in"], dtype=np.float32))
    centers = np.ascontiguousarray(np.asarray(inputs["centers"], dtype=np.float32))
    edge_src = np.asarray(inputs["edge_src"], dtype=np.int32)
    edge_dst = np.asarray(inputs["edge_dst"], dtype=np.int32)

    exp_src, exp_dst = _expected_edges()
    if not (np.array_equal(edge_src, exp_src) and np.array_equal(edge_dst, exp_dst)):
        return _numpy_fallback(
            gnn_in, centers, edge_src, edge_dst,
            {k: np.asarray(v, np.float32) for k, v in inputs.items()
             if k not in ("gnn_in", "centers", "edge_src", "edge_dst")})

    from concourse import bass_utils

    in_maps = _make_in_maps(inputs)
    nc = _get_nc()
    res = bass_utils.run_bass_kernel_spmd(nc, in_maps, core_ids=list(range(N_CORES)))
    out = np.concatenate([r["out_t"] for r in res.results], axis=1)  # [D, N]
    return np.ascontiguousarray(out.T)


def _make_in_maps(inputs) -> list:
    gnn_in = np.ascontiguousarray(np.asarray(inputs["gnn_in"], dtype=np.float32))
    centers = np.ascontiguousarray(np.asarray(inputs["centers"], dtype=np.float32))
    common = {"ind": _build_indicator().astype(np.float16)}
    for li in (1, 2):
        Wf = np.asarray(inputs[f"Wf{li}"], np.float32)
        Ws = np.asarray(inputs[f"Ws{li}"], np.float32)
        common[f"wp{li}"] = np.ascontiguousarray(np.concatenate(
            [Wf[0:D], Wf[D:2 * D], Ws[0:D], Ws[D:2 * D]], axis=1)).astype(np.float16)
        common[f"we{li}"] = np.ascontiguousarray(np.concatenate(
            [Wf[2 * D:], -Wf[2 * D:], Ws[2 * D:], -Ws[2 * D:]], axis=1)).astype(np.float16)
        common[f"bp{li}"] = np.ascontiguousarray(np.stack(
            [np.asarray(inputs[f"bf{li}"], np.float32),
             np.asarray(inputs[f"bs{li}"], np.float32),
             np.asarray(inputs[f"gamma{li}"], np.float32),
             np.asarray(inputs[f"beta{li}"], np.float32)], axis=1))

    in_maps = []
    for c in range(N_CORES):
        sl = slice(c * NODES_PC, (c + 1) * NODES_PC)
        m = dict(common)
        m["xt"] = np.ascontiguousarray(gnn_in[sl].T)
        m["ct"] = np.ascontiguousarray(centers[sl].T).astype(np.float16)
        in_maps.append(m)
    return in_maps



# revision 44
# speedup vs baseline: 1.0271x; 1.0271x over previous
"""Trainium2 Bass kernel for nn_AgentGnn (2-layer CGConv GNN, 128 scenes x 64 agents).

Structure exploited:
- Edges are fully-connected per 64-agent scene (no self loops), so gather/scatter
  becomes dense 64x64 blocks: agg[i] = sum_j sigmoid(F_ij) * softplus(S_ij).
- Per-edge linear terms factor into per-node terms:
    F_ij = af[i] + bf[j],  af = x_i @ Wf[:D] + c_i @ Wf[2D:],
                           bf = x_j @ Wf[D:2D] - c_j @ Wf[2D:]  (+bias)
- sigmoid(F) on ACT from the PE pairwise sums (indicator matmul); scenes are
  processed in table-phase groups with the ACT program order pinned so the
  two activation-table sets (sigmoid / ln+exp) batch their loads.
- softplus(S) = ln(1+e^S) with e^S = e^{as_i} * e^{bs_j}: per-NODE exps
  (tiny ACT passes on weight-stationary matmul outputs in [feature, node]
  layout) combined by one double-broadcast pairwise product per scene -
  mostly on gpsimd (0.42 eff), 1/4 of scenes on DVE (1x broadcast) to
  balance the engines. exp(S) <= ~700 here, far below f16 max. Only ONE
  pairwise ACT pass (Ln) remains on the S side; m = sg * pch on DVE (2x).
- ACT is the bottleneck engine (~81% busy): 2 table passes per pairwise
  element is the floor for this math (no table set holds both a sigmoid-
  family fn and ln; softplus/mish tables don't exist in this overlay).
- j-reduction: all-TT f16 2x halving fold chain (no 1x tensor_reduce).
- The diagonal (self-edge) is removed analytically: for i=j the center
  terms cancel, so F_ii = x@(Wfd+Wfs)+bf (one tiny per-batch matmul) and
  e^{S_ii} = esa*esb; m_ii is subtracted after the j-reduction. No strided
  element accesses anywhere (single-element-stride ops cost 30-150us/op on
  real HW).
- BatchNorm uses exact full stats (per-core/subset stats measure 4-8e-2
  rel err - over the 2e-2 gate); the [128,2] AllReduce per layer is on the
  critical path, so the LAYER TAIL is squeezed: a 3-scene final group,
  halved Ln + halved fold chains there, per-group stats partials with the
  last scene's stats alone, and a fused BN-consts chain.
- Startup: packed weight DMAs (3/layer instead of 12; serial dma_start
  issue on SP costs ~565ns each), xt in halves, batch-1 node terms
  deferred past group 0, and a dummy Exp to hoist the first table load.
- Sharding: 16 scenes (1024 nodes) per core, pure data parallel otherwise.

TimelineSim (single core, fake collective): 327.7us vs 349.6us for the
previous version. Measured on HW (M17 burst marginal): the per-exec fixed
overhead of this runtime path is ~350us on top of the kernel itself
(unroll=2 experiment: marginal 1.09-1.14ms vs 0.74ms at unroll=1).
"""

import numpy as np

N_SAMPLES = 128
AGENTS = 64
D = 128
EDIM = 2
N = N_SAMPLES * AGENTS
EPS = 1e-5

N_CORES = 8
SCENES_PC = N_SAMPLES // N_CORES      # 16 scenes per core
NODES_PC = SCENES_PC * AGENTS         # 1024 nodes per core
PAIR = AGENTS * AGENTS                # 4096 pairwise cols per scene
CHUNK = 2048                          # pairwise chunk (32 i x 64 j)
N_CHUNKS = PAIR // CHUNK              # 2
SUB = 8                               # scenes per node-matmul batch
GRP = 4                               # scenes per ACT-table phase group
DVE_BT_MOD = 4                        # s % DVE_BT_MOD == 0 -> product on DVE
                                      # (gpsimd mult runs at ~0.42 eff; split
                                      # the pairwise e^S products to balance)

_CACHE: dict = {}


def _build_indicator() -> np.ndarray:
    ind = np.zeros((128, PAIR), np.float32)
    for i in range(AGENTS):
        ind[i, i * AGENTS:(i + 1) * AGENTS] = 1.0
    for j in range(AGENTS):
        ind[64 + j, j::AGENTS] = 1.0
    return ind


def _expected_edges():
    a = np.arange(AGENTS)
    rows = np.repeat(a, AGENTS)
    cols = np.tile(a, AGENTS)
    mask = rows != cols
    rows, cols = rows[mask], cols[mask]
    offsets = (np.arange(N_SAMPLES) * AGENTS)[:, None]
    src = (rows[None, :] + offsets).ravel().astype(np.int32)
    dst = (cols[None, :] + offsets).ravel().astype(np.int32)
    return src, dst


def _numpy_fallback(gnn_in, centers, edge_src, edge_dst, ws):
    # generic (slow) reference path, used only if edges don't match the
    # expected block-diagonal fully-connected pattern
    def sigmoid(x):
        return 1.0 / (1.0 + np.exp(-x))

    def softplus(x):
        return np.logaddexp(0.0, x)

    x = gnn_in.astype(np.float64)
    e = (centers[edge_dst] - centers[edge_src]).astype(np.float64)
    for li in (1, 2):
        Wf, bf, Ws, bs, gamma, beta = (ws[f"Wf{li}"], ws[f"bf{li}"], ws[f"Ws{li}"],
                                       ws[f"bs{li}"], ws[f"gamma{li}"], ws[f"beta{li}"])
        z = np.concatenate([x[edge_dst], x[edge_src], e], axis=-1)
        m = sigmoid(z @ Wf + bf) * softplus(z @ Ws + bs)
        agg = np.zeros((N, D))
        np.add.at(agg, edge_dst, m)
        mu = agg.mean(axis=0)
        var = agg.var(axis=0)
        agg = (agg - mu) / np.sqrt(var + EPS) * gamma + beta
        x = np.maximum(agg + x, 0.0)
    return x.astype(np.float32)


def _build_nc(use_collectives=True, unroll=1, stats_subset=False,
              stat_divisor=None, use_gpsimd=True, debug_plain=False,
              palin=False, bt_mod=DVE_BT_MOD, bufs=None, g0sz=4):
    import concourse.bacc as bacc
    import concourse.mybir as mybir
    import concourse.tile as tile

    f32 = mybir.dt.float32
    f16 = mybir.dt.float16
    AF = mybir.ActivationFunctionType
    OP = mybir.AluOpType

    nc = bacc.Bacc("TRN2", target_bir_lowering=False, debug=False,
                   num_devices=N_CORES if use_collectives else 1)

    # ---- I/O ----
    xt_in = nc.dram_tensor("xt", [D, NODES_PC], f32, kind="ExternalInput")
    ct_in = nc.dram_tensor("ct", [EDIM, NODES_PC], f16, kind="ExternalInput")
    ind_in = nc.dram_tensor("ind", [128, PAIR], f16, kind="ExternalInput")
    win = {}
    for li in (1, 2):
        # packed weights: one DMA per pack instead of 12 (startup is gated
        # by serial dma_start issue on the SP sequencer, ~565ns each)
        win[f"wp{li}"] = nc.dram_tensor(f"wp{li}", [D, 4 * D], f16, kind="ExternalInput")
        win[f"we{li}"] = nc.dram_tensor(f"we{li}", [EDIM, 4 * D], f16, kind="ExternalInput")
        win[f"bp{li}"] = nc.dram_tensor(f"bp{li}", [D, 4], f32, kind="ExternalInput")
    out_t = nc.dram_tensor("out_t", [D, NODES_PC], f32, kind="ExternalOutput")

    # nodes the BN statistics are taken over (global, across all cores)
    n_stat = stat_divisor or ((N // 2) if stats_subset else N)

    acts = []  # ACT instructions whose engine order we pin (table batching)

    def act(*args, **kwargs):
        inst = nc.scalar.activation(*args, **kwargs)
        acts.append(inst)
        return inst

    nb = dict(chk=2, pch=5, bt=5, sg=6, ps=2)
    if bufs:
        nb.update(bufs)
    with tile.TileContext(nc) as tc:
        with (
            tc.tile_pool(name="cst", bufs=1) as cst,
            tc.tile_pool(name="wrk", bufs=1) as wrk,
            tc.tile_pool(name="chk", bufs=nb["chk"]) as chk,
            tc.tile_pool(name="pchp", bufs=nb["pch"]) as pchp,
            tc.tile_pool(name="btp", bufs=nb["bt"]) as btp,
            tc.tile_pool(name="sgp", bufs=nb["sg"]) as sgp,
            tc.tile_pool(name="ps", bufs=nb["ps"], space="PSUM") as ps,
            tc.tile_pool(name="dram", bufs=1, space="DRAM") as dram,
        ):
            # ---- load constants ----
            xt = cst.tile([D, NODES_PC], f32)
            ct = cst.tile([EDIM, NODES_PC], f16)
            ind = cst.tile([128, PAIR], f16)
            nc.sync.dma_start(xt[:, 0:NODES_PC // 2], xt_in.ap()[:, 0:NODES_PC // 2])
            nc.sync.dma_start(xt[:, NODES_PC // 2:], xt_in.ap()[:, NODES_PC // 2:])
            wt = {}
            # issue order: layer-1 weights first (startup critical path),
            # then ct/ind (needed at first pairwise matmuls), then layer 2
            for k in ("wp1", "we1", "bp1"):
                h = win[k]
                t = cst.tile(list(h.shape), h.dtype, name=f"t_{k}", tag=f"t_{k}")
                nc.sync.dma_start(t[:], h.ap())
                wt[k] = t
            nc.sync.dma_start(ct[:], ct_in.ap())
            nc.sync.dma_start(ind[:, 0:512], ind_in.ap()[:, 0:512])
            nc.sync.dma_start(ind[:, 512:], ind_in.ap()[:, 512:])
            for k in ("wp2", "we2", "bp2"):
                h = win[k]
                t = cst.tile(list(h.shape), h.dtype, name=f"t_{k}", tag=f"t_{k}")
                nc.sync.dma_start(t[:], h.ap())
                wt[k] = t

            # dummy Exp depending only on the tiny bp1 DMA: hoists the first
            # ln/exp ACT table load to ~t=1us instead of gluing it to the
            # first real exp (which waits on node matmuls)
            warm = wrk.tile([D, 1], f32, name="warm", tag="warm")
            act(warm[:], wt["bp1"][:, 0:1], AF.Exp, bias=0.0, scale=1.0)

            HALF = NODES_PC // 2
            x_carry = xt
            for rep in range(unroll):
              # chain reps through x_carry so unrolled timing builds aren't
              # dead-code eliminated (rep 0 reads the real input)
              x_cur = x_carry
              # fp16 copy of x for the node matmuls (residual stays fp32);
              # done in halves so compute can start before the full DMA
              x16 = wrk.tile([D, NODES_PC], f16, name=f"x16_0r{rep}", tag="x16_0")
              nc.vector.tensor_copy(x16[:, 0:HALF], x_cur[:, 0:HALF])
              nc.vector.tensor_copy(x16[:, HALF:], x_cur[:, HALF:])
              for li0 in (1, 2):
                li = f"{li0}" if rep == 0 else f"{li0}r{rep}"
                par = li0 % 2
                wli = li0
                wp, we, bp = wt[f"wp{wli}"], wt[f"we{wli}"], wt[f"bp{wli}"]
                wfd, wfs = wp[:, 0:D], wp[:, D:2 * D]
                wsd, wss = wp[:, 2 * D:3 * D], wp[:, 3 * D:4 * D]
                wfe, wfen = we[:, 0:D], we[:, D:2 * D]
                wse, wsen = we[:, 2 * D:3 * D], we[:, 3 * D:4 * D]
                bf, bs = bp[:, 0:1], bp[:, 1:2]
                ga, be = bp[:, 2:3], bp[:, 3:4]

                wfds = wrk.tile([D, D], f16, name=f"wfds{li}", tag=f"wfds{par}")
                nc.vector.tensor_tensor(wfds[:], wfd, wfs, OP.add)
                agg = wrk.tile([D, NODES_PC], f32, name=f"agg{li}", tag=f"agg{par}")
                stats = wrk.tile([D, 2], f32, name=f"stats{li}", tag=f"stats{par}")
                # group boundaries: a 3-scene tail group shortens the DVE
                # reduction backlog ahead of the BN stats collective
                GROUPS = [(0, g0sz), (g0sz, 8), (8, 13), (13, 16)]
                NGRP = len(GROUPS)
                GMAX = max(g1 - g0 for g0, g1 in GROUPS)
                sscr = wrk.tile([D, GMAX * AGENTS], f32, name=f"sscr{li}",
                                tag=f"sscr{par}")
                # per-group partials, no serial accumulation chain; one extra
                # slot pair for the split last scene
                sall = wrk.tile([D, 2 * (NGRP + 1)], f32, name=f"sall{li}",
                                tag=f"sall{par}")

                abf = {}
                esa_t, esb_t = {}, {}

                def node_f(batch, bi):
                    # per-scene F-path node terms in [node, feature] layout
                    # (lhsT operand of the pairwise indicator matmul); scenes
                    # pairwise-share a PSUM tile so the first scene's copy
                    # lands after 8 (not 32) matmuls - the layer-boundary
                    # critical path runs through scene 0's abf
                    for k0 in range(0, len(batch), 2):
                        pair = batch[k0:k0 + 2]
                        pab = ps.tile([128, len(pair) * D], f32,
                                      name=f"pab{bi}_{k0}_{li}", tag="pp")
                        for k, s in enumerate(pair):
                            xs = x16[:, s * AGENTS:(s + 1) * AGENTS]
                            cs = ct[:, s * AGENTS:(s + 1) * AGENTS]
                            o = k * D
                            nc.tensor.matmul(pab[0:64, o:o + D], lhsT=cs, rhs=wfe, start=True, stop=False)
                            nc.tensor.matmul(pab[0:64, o:o + D], lhsT=xs, rhs=wfd, start=False, stop=True)
                            nc.tensor.matmul(pab[64:128, o:o + D], lhsT=cs, rhs=wfen, start=True, stop=False)
                            nc.tensor.matmul(pab[64:128, o:o + D], lhsT=xs, rhs=wfs, start=False, stop=True)
                        abt = wrk.tile([128, len(pair) * D], f16,
                                       name=f"ab{bi}_{k0}_{li}", tag=f"ab{bi}_{k0}")
                        nc.vector.tensor_copy(abt[:], pab[:])
                        for k, s in enumerate(pair):
                            abf[s] = abt[:, k * D:(k + 1) * D]

                def node_s(batch, bi):
                    # batched S-path node terms in [feature, node] layout
                    # (weight-stationary), then per-node exp on ACT
                    n0 = batch[0] * AGENTS
                    n = len(batch) * AGENTS
                    pn = ps.tile([128, 2 * n], f32, name=f"pn{bi}_{li}", tag="pp")
                    nc.tensor.matmul(pn[:, 0:n], lhsT=wsd, rhs=x16[:, n0:n0 + n],
                                     start=True, stop=False)
                    nc.tensor.matmul(pn[:, 0:n], lhsT=wse, rhs=ct[:, n0:n0 + n],
                                     start=False, stop=True)
                    nc.tensor.matmul(pn[:, n:2 * n], lhsT=wss, rhs=x16[:, n0:n0 + n],
                                     start=True, stop=False)
                    nc.tensor.matmul(pn[:, n:2 * n], lhsT=wsen, rhs=ct[:, n0:n0 + n],
                                     start=False, stop=True)
                    ea = wrk.tile([D, n], f16, name=f"esa{bi}_{li}", tag=f"esa{bi % 2}")
                    eb = wrk.tile([D, n], f16, name=f"esb{bi}_{li}", tag=f"esb{bi % 2}")
                    act(ea[:], pn[:, 0:n], AF.Exp, bias=bs, scale=1.0)
                    act(eb[:], pn[:, n:2 * n], AF.Exp, bias=0.0, scale=1.0)
                    esa_t[bi] = ea
                    esb_t[bi] = eb
                    # self-edge terms: F_ii = x@(Wfd+Wfs)+bf (centers cancel),
                    # e^{S_ii} = esa*esb. sigmoid/ln are scheduled into the
                    # matching table phases; diag is subtracted after reduce.
                    pnf = ps.tile([128, n], f32, name=f"pnf{bi}_{li}", tag="pp")
                    nc.tensor.matmul(pnf[:, 0:n], lhsT=wfds[:], rhs=x16[:, n0:n0 + n],
                                     start=True, stop=True)
                    # free the PSUM slot right away (sigmoid happens much later);
                    # on DVE to keep the (bottleneck) ACT engine clear
                    fsum = wrk.tile([D, n], f16, name=f"fsum{bi}_{li}", tag=f"fsum{bi % 2}")
                    nc.vector.tensor_copy(fsum[:], pnf[:, 0:n])
                    eii = wrk.tile([D, n], f16, name=f"eii{bi}_{li}", tag=f"eii{bi % 2}")
                    nc.vector.tensor_tensor(eii[:], ea[:], eb[:], OP.mult)
                    dparts[bi] = (fsum, eii)

                sg_map, pch_map = {}, {}
                dparts, dmt = {}, {}

                def diag_sig(bi):
                    # sigmoid(F_ii) - rides in the sigmoid table phase
                    fsum, eii = dparts[bi]
                    n = fsum.shape[1]
                    sii = wrk.tile([D, n], f16, name=f"sii{bi}_{li}", tag=f"sii{bi % 2}")
                    act(sii[:], fsum[:], AF.Sigmoid, bias=bf, scale=1.0)
                    dparts[bi] = (sii, eii)

                def diag_ln(bi):
                    # ln(1+e^{S_ii}) - rides in the ln/exp table phase
                    sii, eii = dparts[bi]
                    n = eii.shape[1]
                    spd = wrk.tile([D, n], f16, name=f"spd{bi}_{li}", tag=f"spd{bi % 2}")
                    act(spd[:], eii[:], AF.Ln, bias=1.0, scale=1.0)
                    dparts[bi] = (sii, spd)

                def diag_dm(bi):
                    # m_ii = sigmoid(F_ii) * ln(1+e^{S_ii}) on DVE
                    sii, spd = dparts[bi]
                    n = spd.shape[1]
                    dm = wrk.tile([D, n], f16, name=f"dm{bi}_{li}", tag=f"dm{bi % 2}")
                    nc.vector.tensor_tensor(dm[:], sii[:], spd[:], OP.mult)
                    dmt[bi] = dm

                def scene_sig(s):
                    # F-path: pairwise sums on PE, sigmoid on ACT
                    sg = sgp.tile([D, PAIR], f16, name=f"sg{s}", tag="sg")
                    for c in range(N_CHUNKS):
                        pf = ps.tile([D, CHUNK], f32, name=f"pf{s}_{c}", tag="pp")
                        for k in range(CHUNK // 512):
                            col = c * CHUNK + k * 512
                            nc.tensor.matmul(pf[:, k * 512:(k + 1) * 512],
                                             lhsT=abf[s][:],
                                             rhs=ind[:, col:col + 512],
                                             start=True, stop=True)
                        act(sg[:, c * CHUNK:(c + 1) * CHUNK], pf[:],
                            AF.Sigmoid, bias=bf, scale=1.0)
                    sg_map[s] = sg

                def scene_bt(s, bi):
                    ea, eb = esa_t[bi], esb_t[bi]
                    o = (s % SUB) * AGENTS
                    ea_b = ea[:, o:o + AGENTS].unsqueeze(2).broadcast_to(
                        (D, AGENTS, AGENTS))
                    eb_b = eb[:, o:o + AGENTS].unsqueeze(1).broadcast_to(
                        (D, AGENTS, AGENTS))
                    # pairwise e^S = e^{as_i} * e^{bs_j} (diag handled
                    # analytically afterwards); split across DVE (1x, broadcast
                    # APs) and gpsimd (0.42 eff) to balance the two engines.
                    # Tail-group scenes stay OFF the DVE: its backlog there
                    # feeds the BN-collective critical path.
                    bt = btp.tile([D, PAIR], f16, name="bt", tag="bt")
                    bt3 = bt.rearrange("p (i j) -> p i j", j=AGENTS)
                    on_dve = (s % bt_mod == 2 % bt_mod) and s < GROUPS[-1][0]
                    if use_gpsimd and not on_dve:
                        nc.gpsimd.tensor_tensor(bt3, ea_b, eb_b, OP.mult)
                    else:
                        nc.vector.tensor_tensor(bt3, ea_b, eb_b, OP.mult)
                    return bt

                LAST = SCENES_PC - 1

                def scene_ln(s, bt):
                    # softplus = ln(1 + e^S) - the only pairwise S-path ACT op.
                    # The LAST scene of the layer is split into i-halves so the
                    # DVE reduction (critical path into the BN collective)
                    # starts ~2us earlier.
                    pch = pchp.tile([D, PAIR], f16, name="pch", tag="pch")
                    if s >= GROUPS[-1][0]:
                        act(pch[:, 0:PAIR // 2], bt[:, 0:PAIR // 2],
                            AF.Ln, bias=1.0, scale=1.0)
                        act(pch[:, PAIR // 2:], bt[:, PAIR // 2:],
                            AF.Ln, bias=1.0, scale=1.0)
                    else:
                        act(pch[:], bt[:], AF.Ln, bias=1.0, scale=1.0)
                    pch_map[s] = pch

                def red_part(s, bi, pch, sg, h, nh):
                    # m = sigmoid(F) * softplus(S) on DVE (f16 2x), then
                    # j-reduction: all-TT halving fold chain (f16 2x; avoids
                    # the 1x-rate tensor_reduce). h/nh select an i-chunk.
                    ni = AGENTS // nh
                    sl = slice(h * ni * AGENTS, (h + 1) * ni * AGENTS)
                    nc.vector.tensor_tensor(pch[:, sl], sg[:, sl], pch[:, sl],
                                            OP.mult)
                    cur = pch[:, sl].rearrange("p (i j) -> p i j", j=AGENTS)
                    w = AGENTS
                    fi = 0
                    while w > 2:
                        w //= 2
                        fi += 1
                        fd = chk.tile([D, ni * w], f16, name=f"fd{fi}",
                                      tag=f"fd{fi}")
                        f3 = fd.rearrange("p (i j) -> p i j", j=w)
                        nc.vector.tensor_tensor(f3, cur[:, :, 0:w], cur[:, :, w:2 * w],
                                                OP.add)
                        cur = f3
                    asl = agg[:, s * AGENTS + h * ni:s * AGENTS + (h + 1) * ni]
                    nc.vector.tensor_tensor(
                        asl.rearrange("p (i j) -> p i j", j=1),
                        cur[:, :, 0:1], cur[:, :, 1:2], OP.add)
                    o = (s % SUB) * AGENTS + h * ni
                    nc.vector.tensor_tensor(
                        asl, asl, dmt[bi][:, o:o + ni], OP.subtract)

                def stats_part(slot, n0, n1):
                    # BN partial stats (sum + sumsq) over node cols [n0, n1)
                    gsl = agg[:, n0:n1]
                    nc.vector.tensor_tensor(sscr[:, 0:n1 - n0], gsl, gsl, OP.mult)
                    nc.vector.tensor_reduce(sall[:, 2 * slot:2 * slot + 1], gsl,
                                            axis=mybir.AxisListType.X, op=OP.add)
                    nc.vector.tensor_reduce(sall[:, 2 * slot + 1:2 * slot + 2],
                                            sscr[:, 0:n1 - n0],
                                            axis=mybir.AxisListType.X, op=OP.add)

                def scene_red(s, bi, gi, g0, g1):
                    pch = pch_map.pop(s)
                    sg = sg_map.pop(s)
                    if gi == NGRP - 1:
                        # tail group: halved reductions overlap the (also
                        # halved) Ln ops - shortest chain into the collective
                        red_part(s, bi, pch, sg, 0, 2)
                        red_part(s, bi, pch, sg, 1, 2)
                    else:
                        red_part(s, bi, pch, sg, 0, 1)
                    if s == LAST:
                        # last scene's stats alone (the rest of its group was
                        # already folded in at s-1) - shortest possible tail
                        stats_part(NGRP, s * AGENTS, (s + 1) * AGENTS)
                    elif s == g1 - 1 and g1 != SCENES_PC:
                        stats_part(gi, g0 * AGENTS, g1 * AGENTS)
                    elif s == LAST - 1:
                        # tail group's scenes ahead of the last scene
                        stats_part(gi, g0 * AGENTS, (s + 1) * AGENTS)

                def bn_consts():
                    # combine per-group partials: view [d, c(2), g] with
                    # c outer (stride 1) and g inner (stride 2), reduce g
                    sview = sall[:, 0:2 * (NGRP + 1)].rearrange(
                        "p (s c) -> p c s", c=2)
                    nc.vector.tensor_reduce(
                        stats.rearrange("p (c o) -> p c o", o=1), sview,
                        axis=mybir.AxisListType.X, op=OP.add)
                    # AllReduce the [sum, sumsq] stats, then fold into A, B
                    cc_in = dram.tile([D, 2], f32, name=f"ccin{li}", tag=f"ccin{li}")
                    cc_out = dram.tile([D, 2], f32, name=f"ccout{li}", tag=f"ccout{li}",
                                       addr_space="Shared")
                    nc.sync.dma_start(cc_in[:], stats[:])
                    if use_collectives:
                        nc.gpsimd.collective_compute(
                            "AllReduce", OP.add,
                            replica_groups=[list(range(N_CORES))],
                            ins=[cc_in.opt()], outs=[cc_out.opt()])
                    else:
                        nc.sync.dma_start(cc_out[:], cc_in[:])
                    stot = wrk.tile([D, 2], f32, name=f"stot{li}", tag="stot")
                    nc.sync.dma_start(stot[:], cc_out[:])
                    # me = [mu, ex2]; rstd = exp(-0.5*ln(var+eps));
                    # A = gamma*rstd; B = beta - mu*A (shortest serial chain)
                    me = wrk.tile([D, 2], f32, name="me", tag="me")
                    nc.vector.tensor_scalar_mul(me[:], stot[:], 1.0 / n_stat)
                    var = wrk.tile([D, 1], f32, name="var", tag="var")
                    nc.vector.tensor_tensor(var[:], me[:, 0:1], me[:, 0:1], OP.mult)
                    nc.vector.scalar_tensor_tensor(var[:], me[:, 1:2], EPS,
                                                   var[:], OP.add, OP.subtract)
                    rstd = wrk.tile([D, 1], f32, name="rstd", tag="rstd")
                    nc.scalar.activation(rstd[:], var[:], AF.Ln, bias=0.0, scale=1.0)
                    nc.scalar.activation(rstd[:], rstd[:], AF.Exp, bias=0.0, scale=-0.5)
                    A = wrk.tile([D, 1], f32, name="A", tag="A")
                    Bt = wrk.tile([D, 1], f32, name="Bt", tag="Bt")
                    nc.vector.tensor_tensor(A[:], ga, rstd[:], OP.mult)
                    nc.vector.tensor_tensor(Bt[:], me[:, 0:1], A[:], OP.mult)
                    nc.vector.tensor_tensor(Bt[:], be, Bt[:], OP.subtract)
                    return A, Bt

                batches = [list(range(b0, min(b0 + SUB, SCENES_PC)))
                           for b0 in range(0, SCENES_PC, SUB)]
                node_s(batches[0], 0)
                node_f(batches[0], 0)
                ar_done = [False]

                def run_group(gi, g0, g1):
                    # palindromic ACT phase order: even groups run
                    # [sigmoid-table phase, ln-table phase], odd groups
                    # [ln, sigmoid] - adjacent same-set phases merge, so
                    # table loads drop from 2/group to ~1/group. The ln
                    # phase never depends on the sigmoid phase (the m
                    # product on DVE joins them afterwards).
                    bts = {s: scene_bt(s, s // SUB) for s in range(g0, g1)}
                    dbis = [bi for bi in (0, 1) if g0 <= bi * SUB < g1]

                    def sig_phase():
                        for bi in dbis:
                            diag_sig(bi)
                        for s in range(g0, g1):
                            scene_sig(s)

                    def ln_phase():
                        for bi in dbis:
                            diag_ln(bi)
                        for s in range(g0, g1):
                            scene_ln(s, bts[s])

                    if (gi % 2 == 0) or not palin:
                        sig_phase()
                        ln_phase()
                    else:
                        ln_phase()
                        sig_phase()
                    for bi in dbis:
                        diag_dm(bi)
                    for s in range(g0, g1):
                        scene_red(s, s // SUB, gi, g0, g1)

                for gi, (g0, g1) in enumerate(GROUPS):
                    if g0 == SUB:
                        # batch-1 node terms deferred past the early groups:
                        # keeps the two PSUM slots free during startup
                        # (batch-0 exps release them before group 0's
                        # pairwise matmuls), and the Exp ops ride the
                        # adjacent ln-set phase
                        node_s(batches[1], 1)
                        node_f(batches[1], 1)
                    run_group(gi, g0, g1)

                # x_next = relu(agg*A + B + x_cur) applied per half so the
                # next layer's node matmuls (which only need one half) can
                # start while this layer's second half is still draining
                xn = wrk.tile([D, NODES_PC], f32, name=f"x{li}", tag=f"xn{par}")
                x16n = None
                if li0 == 1 or rep + 1 < unroll:
                    x16n = wrk.tile([D, NODES_PC], f16, name=f"x16_{li}",
                                    tag=f"x16_{li0 % 2}")

                def apply_half(h):
                    sl = slice(h * HALF, (h + 1) * HALF)
                    nc.vector.scalar_tensor_tensor(xn[:, sl], agg[:, sl],
                                                   A[:, 0:1], x_cur[:, sl],
                                                   OP.mult, OP.add)
                    nc.vector.tensor_scalar(xn[:, sl], xn[:, sl], Bt[:, 0:1],
                                            0.0, OP.add, OP.max)
                    if x16n is not None:
                        nc.vector.tensor_copy(x16n[:, sl], xn[:, sl])

                A, Bt = bn_consts()
                apply_half(0)
                apply_half(1)
                x_cur = xn
                x16 = x16n
              x_carry = x_cur

            nc.sync.dma_start(out_t.ap()[:, 0:HALF], x_cur[:, 0:HALF])
            nc.sync.dma_start(out_t.ap()[:, HALF:], x_cur[:, HALF:])

        from concourse.tile_rust import add_dep_helper
        for a, b in zip(acts, acts[1:]):
            add_dep_helper(b.ins, a.ins, sync=False,
                           reason="ACT table-set batching order")

    # Restrict the act-table chooser so Exp and Ln resolve to the shared
    # natural_log_exp set (all ACT ops here live in that one set; the
    # default chooser could otherwise alternate sets and thrash ~2.7us
    # table loads).
    keep = {"sigmoid_and_others", "natural_log_exp_and_others"}
    orig_tables = bacc.get_activation_tables

    def patched_tables(arch):
        return {k: (v if k in keep else set())
                for k, v in orig_tables(arch).items()}

    bacc.get_activation_tables = patched_tables
    try:
        nc.compile()
    finally:
        bacc.get_activation_tables = orig_tables
    return nc


def _get_nc():
    if "nc" not in _CACHE:
        _CACHE["nc"] = _build_nc()
    return _CACHE["nc"]


def kernel(**inputs) -> np.ndarray:
    gnn_in = np.ascontiguousarray(np.asarray(inputs["gnn_in"], dtype=np.float32))
    centers = np.ascontiguousarray(np.asarray(inputs["centers"], dtype=np.float32))
    edge_src = np.asarray(inputs["edge_src"], dtype=np.int32)
    edge_dst = np.asarray(inputs["edge_dst"], dtype=np.int32)

    exp_src, exp_dst = _expected_edges()
    if not (np.array_equal(edge_src, exp_src) and np.array_equal(edge_dst, exp_dst)):
        return _numpy_fallback(
            gnn_in, centers, edge_src, edge_dst,
            {k: np.asarray(v, np.float32) for k, v in inputs.items()
             if k not in ("gnn_in", "centers", "edge_src", "edge_dst")})

    from concourse import bass_utils

    in_maps = _make_in_maps(inputs)
    nc = _get_nc()
    res = bass_utils.run_bass_kernel_spmd(nc, in_maps, core_ids=list(range(N_CORES)))
    out = np.concatenate([r["out_t"] for r in res.results], axis=1)  # [D, N]
    return np.ascontiguousarray(out.T)


def _make_in_maps(inputs) -> list:
    gnn_in = np.ascontiguousarray(np.asarray(inputs["gnn_in"], dtype=np.float32))
    centers = np.ascontiguousarray(np.asarray(inputs["centers"], dtype=np.float32))
    common = {"ind": _build_indicator().astype(np.float16)}
    for li in (1, 2):
        Wf = np.asarray(inputs[f"Wf{li}"], np.float32)
        Ws = np.asarray(inputs[f"Ws{li}"], np.float32)
        common[f"wp{li}"] = np.ascontiguousarray(np.concatenate(
            [Wf[0:D], Wf[D:2 * D], Ws[0:D], Ws[D:2 * D]], axis=1)).astype(np.float16)
        common[f"we{li}"] = np.ascontiguousarray(np.concatenate(
            [Wf[2 * D:], -Wf[2 * D:], Ws[2 * D:], -Ws[2 * D:]], axis=1)).astype(np.float16)
        common[f"bp{li}"] = np.ascontiguousarray(np.stack(
            [np.asarray(inputs[f"bf{li}"], np.float32),
             np.asarray(inputs[f"bs{li}"], np.float32),
             np.asarray(inputs[f"gamma{li}"], np.float32),
             np.asarray(inputs[f"beta{li}"], np.float32)], axis=1))

    in_maps = []
    for c in range(N_CORES):
        sl = slice(c * NODES_PC, (c + 1) * NODES_PC)
        m = dict(common)
        m["xt"] = np.ascontiguousarray(gnn_in[sl].T)
        m["ct"] = np.ascontiguousarray(centers[sl].T).astype(np.float16)
        in_maps.append(m)
    return in_maps

